# revision 14
# baseline (speedup 1.0000x reference)
"""MoE GroupedExperts kernel for 8 TRN2 NeuronCores.

Expert-parallel: expert e's tokens + weights go to core e. Tokens are
pre-sorted by expert, so routing is host-side slicing. Each core runs a
SwiGLU MLP: o = (silu(x @ gate) * (x @ up)) @ down.

The kernel is at the compute/memory ridge: per-core weight traffic
(12.6MB fp16) at ~290 GB/s takes ~44us, and the PE GEMM stream takes
~42us. The design goal is therefore near-perfect DMA/PE overlap:

- All tensors are packed on the host into the exact SBUF layout
  (partition-major) so every DMA descriptor is a maximal contiguous run.
- DMA triggers are issued at t~=0 (the baseline lost ~7us to preamble
  ordering), split across both HWDGE rings in consumption order, with
  down_proj chunks interleaved into the stream.
- Compute is a chunk pipeline: per 256-col hid chunk g, the gate/up
  GEMMs for chunk g run, then the down-proj partial GEMMs for chunk g-1
  (one chunk of slack so a late dw chunk never head-of-line-blocks the
  PE queue).
- Dummy matmuls at t=0 warm the PE HAM clock gate (idle default is
  1.2 GHz; ~3.4us of activity unlocks 2.4 GHz).
- Output DMA goes out on the gpsimd SWDGE queue, off both input rings.
"""

import sys

if "/opt/trn_rl_repo" not in sys.path:
    sys.path.insert(0, "/opt/trn_rl_repo")

import numpy as np

F16 = np.float16
E = 8
DIM = 1024
HID = 2048
N_CORES = 8
CPAD = 256          # tokens per expert per block (T/E for the target shape)
KC = DIM // 128     # 8 k-chunks for gate/up contraction
KH = HID // 128     # 16 k-chunks for down contraction
NCH = HID // 256    # 8 hid chunks (256 cols each)
CH = 256            # hid cols per chunk
PAIR = 2            # hid slices per PSUM bank (2*256 = 512 fp32)

_cache = {}


def _build():
    from concourse import bacc
    import concourse.tile as tile
    import concourse.mybir as mybir

    f32 = mybir.dt.float32
    f16 = mybir.dt.float16

    nc = bacc.Bacc("TRN2", target_bir_lowering=False, debug=False)
    # Packed DRAM layouts: partition dim first, contiguous per partition.
    xt_d = nc.dram_tensor("xt", [128, KC * CPAD], f16, kind="ExternalInput")
    gu_d = nc.dram_tensor("gu", [128, NCH * 2 * KC * CH], f16, kind="ExternalInput")
    dw_d = nc.dram_tensor("dw", [128, KH * DIM], f16, kind="ExternalInput")
    o_d = nc.dram_tensor("o", [CPAD, DIM], f16, kind="ExternalOutput")

    NTOK = CPAD // 128  # 2 token tiles
    NDC = DIM // 512    # 2 output column slices

    with tile.TileContext(nc) as tc:
        with (
            tc.tile_pool(name="sb", bufs=1) as sb,
            tc.tile_pool(name="stmp", bufs=2) as stmp_pool,
            tc.tile_pool(name="ht", bufs=3) as ht_pool,
            tc.tile_pool(name="outp", bufs=2) as out_pool,
            tc.tile_pool(name="psA", bufs=2, space="PSUM") as psA,
            tc.tile_pool(name="psB", bufs=2, space="PSUM") as psB,
            tc.tile_pool(name="psO", bufs=4, space="PSUM") as psO,
        ):
            xt_s = sb.tile([128, KC, CPAD], f16)
            gu_s = sb.tile([128, NCH, 2, KC * CH], f16)
            dw_s = sb.tile([128, KH, DIM], f16)
            warm = sb.tile([128, 512], f16)

            # --- HAM warmup: dummy matmuls so the PE clock is at 2.4 GHz
            # by the time real data lands (~3.4us of PE activity needed).
            nc.gpsimd.memset(warm[:], 0)
            wps = psA.tile([128, PAIR, CPAD], f32, tag="pg", name="warmps")
            for i in range(24):
                nc.tensor.matmul(
                    wps[:, i % PAIR, :], warm[:, 0:128], warm[:, 0:CPAD],
                    start=True, stop=True, skip_group_check=True,
                )

            # --- DMA triggers. Two concurrent channels (measured ~250-300
            # GB/s each, ~500 aggregate): the sync HWDGE ring and the
            # gpsimd software-DGE queue. The scalar engine queue stays
            # free for silu so the PSUM recycle chain never gates the PE
            # (the compiler also parks the act-table loads at the head of
            # the scalar queue, which would delay any transfer behind
            # them by ~2.5us). Every transfer is a contiguous range of
            # the packed layout; the first x/gate chunks are split on the
            # k axis (contiguous) so the PE's first matmul can start
            # ~3us earlier.
            # sync ring:  x (2 halves), gate0 (2 halves), up0, then full
            #             gate+up chunks 1,2,4,6.
            # gpsimd:     down chunks (rows 14,15 singly so the last
            #             arrival feeds only the last matmuls), with
            #             gate+up chunks 3,5,7 interleaved by need time.
            xt_v = xt_d.ap()
            gu_v = gu_d.ap()
            dw_v = dw_d.ap()
            GUC = 2 * KC * CH  # elements per gate+up chunk per partition

            def dma_gu(g, eng):
                c0 = g * GUC
                eng.dma_start(gu_s[:, g, :, :], gu_v[:, c0:c0 + GUC])

            xt_3d = xt_v.rearrange("p (k c) -> p k c", k=KC)
            h = KC // 2
            nc.sync.dma_start(xt_s[:, 0:h, :], xt_3d[:, 0:h, :])
            nc.sync.dma_start(xt_s[:, h:, :], xt_3d[:, h:, :])
            g0 = gu_s[:, 0, :, :].rearrange("p m (k c) -> p (m k) c", k=KC)
            g0v = gu_v.rearrange("p (g m k c) -> p g (m k) c", g=NCH, m=2, k=KC)
            nc.sync.dma_start(g0[:, 0:h, :], g0v[:, 0, 0:h, :])
            nc.sync.dma_start(g0[:, h:KC, :], g0v[:, 0, h:KC, :])
            nc.sync.dma_start(g0[:, KC:, :], g0v[:, 0, KC:, :])
            # remaining chunks, one queue, in exact consumption order:
            # down rows 2g-2,2g-1 land just before gate/up chunk g, so
            # the down(g-1) matmuls (which directly follow gate_up(g-1)
            # on the PE) are always fed; final dw rows ride singly so
            # the last arrival feeds only the last 4 matmuls.
            for g in range(1, NCH):
                k0, k1 = 2 * (g - 1), 2 * g
                nc.sync.dma_start(
                    dw_s[:, k0:k1, :], dw_v[:, k0 * DIM:k1 * DIM]
                )
                dma_gu(g, nc.sync)
            for k0, k1 in [(12, 14), (14, 15), (15, 16)]:
                nc.sync.dma_start(
                    dw_s[:, k0:k1, :], dw_v[:, k0 * DIM:k1 * DIM]
                )

            # --- chunk pipeline
            po = [
                psO.tile([128, 512], f32, tag="po", name=f"po{tok}_{dc}")
                for tok in range(NTOK) for dc in range(NDC)
            ]
            ht = [None] * NCH

            def gate_up(g):
                pg = psA.tile([128, PAIR, CPAD], f32, tag="pg")
                pu = psB.tile([128, PAIR, CPAD], f32, tag="pu")
                ht[g] = ht_pool.tile([128, PAIR, CPAD], f16, tag="ht", name=f"ht{g}")
                for j in range(PAIR):
                    cj = j * 128
                    for k in range(KC):
                        nc.tensor.matmul(
                            pg[:, j, :],
                            gu_s[:, g, 0, k * CH + cj:k * CH + cj + 128],
                            xt_s[:, k, :],
                            start=(k == 0), stop=(k == KC - 1),
                            skip_group_check=True,
                        )
                    for k in range(KC):
                        nc.tensor.matmul(
                            pu[:, j, :],
                            gu_s[:, g, 1, k * CH + cj:k * CH + cj + 128],
                            xt_s[:, k, :],
                            start=(k == 0), stop=(k == KC - 1),
                            skip_group_check=True,
                        )
                    stmp = stmp_pool.tile([128, CPAD], f32, tag="stmp",
                                          name=f"stmp{g}_{j}")
                    nc.scalar.activation(
                        stmp[:], pg[:, j, :], mybir.ActivationFunctionType.Silu
                    )
                    nc.vector.tensor_mul(ht[g][:, j, :], stmp[:], pu[:, j, :])

            def down(g):
                for k in (2 * g, 2 * g + 1):
                    for tok in range(NTOK):
                        t0, t1 = tok * 128, (tok + 1) * 128
                        for dc in range(NDC):
                            nc.tensor.matmul(
                                po[tok * NDC + dc][:],
                                ht[g][:, k % PAIR, t0:t1],
                                dw_s[:, k, dc * 512:(dc + 1) * 512],
                                start=(k == 0), stop=(k == KH - 1),
                                skip_group_check=True,
                            )

            for g in range(NCH):
                gate_up(g)
                if g < NCH - 1:
                    down(g)

            # Last down chunk, regrouped per chain: each chain's final
            # matmuls are followed immediately by its PSUM->fp16 cast and
            # output-quarter DMA, so they overlap the other chains' tail
            # matmuls and only ~1us of work remains after the last MM.
            g = NCH - 1
            out_tiles = [
                out_pool.tile([128, DIM], f16, tag="out", name=f"out{t}")
                for t in range(NTOK)
            ]
            for tok in range(NTOK):
                t0, t1 = tok * 128, (tok + 1) * 128
                for dc in range(NDC):
                    for k in (2 * g, 2 * g + 1):
                        nc.tensor.matmul(
                            po[tok * NDC + dc][:],
                            ht[g][:, k % PAIR, t0:t1],
                            dw_s[:, k, dc * 512:(dc + 1) * 512],
                            start=False, stop=(k == KH - 1),
                            skip_group_check=True,
                        )
                    nc.vector.tensor_copy(
                        out_tiles[tok][:, dc * 512:(dc + 1) * 512],
                        po[tok * NDC + dc][:],
                    )
                    nc.sync.dma_start(
                        o_d[t0:t1, dc * 512:(dc + 1) * 512],
                        out_tiles[tok][:, dc * 512:(dc + 1) * 512],
                    )

    nc.compile()
    return nc


def _get_nc():
    if "nc" not in _cache:
        _cache["nc"] = _build()
    return _cache["nc"]


def _pack_x(xe):
    # xe [CPAD, DIM] fp16 -> [128, KC*CPAD]: [p][k][c], dim = k*128+p
    return np.ascontiguousarray(
        xe.T.reshape(KC, 128, CPAD).transpose(1, 0, 2).reshape(128, KC * CPAD)
    )


def _pack_gu(gw, uw):
    # [DIM, HID] x2 fp16 -> [128, NCH*2*KC*CH]: [p][g][m][k][c], m=gate/up
    a = gw.reshape(KC, 128, NCH, CH).transpose(1, 2, 0, 3)  # [p][g][k][c]
    b = uw.reshape(KC, 128, NCH, CH).transpose(1, 2, 0, 3)
    return np.ascontiguousarray(
        np.stack([a, b], axis=2).reshape(128, -1)
    )


def _pack_dw(w):
    # w [HID, DIM] fp16 -> [128, KH*DIM]: [p][k][d]
    return np.ascontiguousarray(
        w.reshape(KH, 128, DIM).transpose(1, 0, 2).reshape(128, -1)
    )


def _run_block(nc, in_maps, collect):
    from concourse.bass_utils import run_bass_kernel_spmd

    kwargs = {} if collect is None else dict(collect.get("run_kwargs") or {})
    res = run_bass_kernel_spmd(nc, in_maps, core_ids=list(range(N_CORES)), **kwargs)
    if collect is not None:
        collect.setdefault("results", []).append(res)
    return [res.results[e]["o"] for e in range(E)]


def kernel(x, counts, gate_proj, up_proj, down_proj, _collect=None):
    x = np.asarray(x, dtype=np.float32).astype(F16)
    counts = np.asarray(counts, dtype=np.int32)
    gate_proj = np.asarray(gate_proj, dtype=np.float32).astype(F16)
    up_proj = np.asarray(up_proj, dtype=np.float32).astype(F16)
    down_proj = np.asarray(down_proj, dtype=np.float32).astype(F16)

    T = x.shape[0]
    offs = np.concatenate([[0], np.cumsum(counts)]).astype(np.int64)
    cmax = int(counts.max()) if counts.size else CPAD
    n_blocks = max(1, -(-cmax // CPAD))

    nc = _get_nc()
    wpacks = [
        {
            "gu": _pack_gu(gate_proj[e], up_proj[e]),
            "dw": _pack_dw(down_proj[e]),
        }
        for e in range(E)
    ]

    out = np.empty((T, DIM), dtype=np.float32)
    for b in range(n_blocks):
        in_maps = []
        spans = []
        for e in range(E):
            c = int(counts[e])
            s0 = min(b * CPAD, c)
            s1 = min((b + 1) * CPAD, c)
            xe = x[offs[e] + s0:offs[e] + s1]
            if xe.shape[0] < CPAD:
                xe = np.concatenate(
                    [xe, np.zeros((CPAD - xe.shape[0], DIM), F16)], axis=0
                )
            in_maps.append({"xt": _pack_x(xe), **wpacks[e]})
            spans.append((s0, s1))
        outs = _run_block(nc, in_maps, _collect)
        for e in range(E):
            s0, s1 = spans[e]
            if s1 > s0:
                out[offs[e] + s0:offs[e] + s1] = outs[e][: s1 - s0]
    return out


# revision 15
# speedup vs baseline: 1.0163x; 1.0163x over previous
"""MoE GroupedExperts kernel for 8 TRN2 NeuronCores.

Expert-parallel: expert e's tokens + weights go to core e. Tokens are
pre-sorted by expert, so routing is host-side slicing. Each core runs a
SwiGLU MLP: o = (silu(x @ gate) * (x @ up)) @ down.

The kernel is at the compute/memory ridge: per-core weight traffic
(12.6MB fp16) at ~290 GB/s takes ~44us, and the PE GEMM stream takes
~42us. The design goal is therefore near-perfect DMA/PE overlap:

- All tensors are packed on the host into the exact SBUF layout
  (partition-major) so every DMA descriptor is a maximal contiguous run.
- DMA triggers are issued at t~=0 (the baseline lost ~7us to preamble
  ordering), split across both HWDGE rings in consumption order, with
  down_proj chunks interleaved into the stream.
- Compute is a chunk pipeline: per 256-col hid chunk g, the gate/up
  GEMMs for chunk g run, then the down-proj partial GEMMs for chunk g-1
  (one chunk of slack so a late dw chunk never head-of-line-blocks the
  PE queue).
- Dummy matmuls at t=0 warm the PE HAM clock gate (idle default is
  1.2 GHz; ~3.4us of activity unlocks 2.4 GHz).
- Output DMA goes out on the gpsimd SWDGE queue, off both input rings.
"""

import sys

if "/opt/trn_rl_repo" not in sys.path:
    sys.path.insert(0, "/opt/trn_rl_repo")

import numpy as np

F16 = np.float16
E = 8
DIM = 1024
HID = 2048
N_CORES = 8
CPAD = 256          # tokens per expert per block (T/E for the target shape)
KC = DIM // 128     # 8 k-chunks for gate/up contraction
KH = HID // 128     # 16 k-chunks for down contraction
NCH = HID // 256    # 8 hid chunks (256 cols each)
CH = 256            # hid cols per chunk
PAIR = 2            # hid slices per PSUM bank (2*256 = 512 fp32)

_cache = {}


def _build():
    from concourse import bacc
    import concourse.tile as tile
    import concourse.mybir as mybir

    f32 = mybir.dt.float32
    f16 = mybir.dt.float16

    nc = bacc.Bacc("TRN2", target_bir_lowering=False, debug=False)
    # Packed DRAM layouts: partition dim first, contiguous per partition.
    xt_d = nc.dram_tensor("xt", [128, KC * CPAD], f16, kind="ExternalInput")
    gu_d = nc.dram_tensor("gu", [128, NCH * 2 * KC * CH], f16, kind="ExternalInput")
    dw_d = nc.dram_tensor("dw", [128, KH * DIM], f16, kind="ExternalInput")
    o_d = nc.dram_tensor("o", [CPAD, DIM], f16, kind="ExternalOutput")

    NTOK = CPAD // 128  # 2 token tiles
    NDC = DIM // 512    # 2 output column slices

    with tile.TileContext(nc) as tc:
        with (
            tc.tile_pool(name="sb", bufs=1) as sb,
            tc.tile_pool(name="stmp", bufs=2) as stmp_pool,
            tc.tile_pool(name="ht", bufs=3) as ht_pool,
            tc.tile_pool(name="outp", bufs=2) as out_pool,
            tc.tile_pool(name="psA", bufs=2, space="PSUM") as psA,
            tc.tile_pool(name="psB", bufs=2, space="PSUM") as psB,
            tc.tile_pool(name="psO", bufs=4, space="PSUM") as psO,
        ):
            xt_s = sb.tile([128, KC, CPAD], f16)
            gu_s = sb.tile([128, NCH, 2, KC * CH], f16)
            dw_s = sb.tile([128, KH, DIM], f16)
            warm = sb.tile([128, 512], f16)

            # --- HAM warmup: dummy matmuls so the PE clock is at 2.4 GHz
            # by the time real data lands (~3.4us of PE activity needed).
            nc.gpsimd.memset(warm[:], 0)
            wps = psA.tile([128, PAIR, CPAD], f32, tag="pg", name="warmps")
            for i in range(24):
                nc.tensor.matmul(
                    wps[:, i % PAIR, :], warm[:, 0:128], warm[:, 0:CPAD],
                    start=True, stop=True, skip_group_check=True,
                )

            # --- DMA triggers. Two concurrent channels (measured ~250-300
            # GB/s each, ~500 aggregate): the sync HWDGE ring and the
            # gpsimd software-DGE queue. The scalar engine queue stays
            # free for silu so the PSUM recycle chain never gates the PE
            # (the compiler also parks the act-table loads at the head of
            # the scalar queue, which would delay any transfer behind
            # them by ~2.5us). Every transfer is a contiguous range of
            # the packed layout; the first x/gate chunks are split on the
            # k axis (contiguous) so the PE's first matmul can start
            # ~3us earlier.
            # sync ring:  x (2 halves), gate0 (2 halves), up0, then full
            #             gate+up chunks 1,2,4,6.
            # gpsimd:     down chunks (rows 14,15 singly so the last
            #             arrival feeds only the last matmuls), with
            #             gate+up chunks 3,5,7 interleaved by need time.
            xt_v = xt_d.ap()
            gu_v = gu_d.ap()
            dw_v = dw_d.ap()
            GUC = 2 * KC * CH  # elements per gate+up chunk per partition

            def dma_gu(g, eng):
                c0 = g * GUC
                eng.dma_start(gu_s[:, g, :, :], gu_v[:, c0:c0 + GUC])

            xt_3d = xt_v.rearrange("p (k c) -> p k c", k=KC)
            h = KC // 2
            nc.sync.dma_start(xt_s[:, 0:h, :], xt_3d[:, 0:h, :])
            nc.sync.dma_start(xt_s[:, h:, :], xt_3d[:, h:, :])
            g0 = gu_s[:, 0, :, :].rearrange("p m (k c) -> p (m k) c", k=KC)
            g0v = gu_v.rearrange("p (g m k c) -> p g (m k) c", g=NCH, m=2, k=KC)
            nc.sync.dma_start(g0[:, 0:h, :], g0v[:, 0, 0:h, :])
            nc.sync.dma_start(g0[:, h:KC, :], g0v[:, 0, h:KC, :])
            nc.sync.dma_start(g0[:, KC:, :], g0v[:, 0, KC:, :])
            # remaining gate/up chunks and down chunks, one queue, in
            # exact consumption order (gu_g at chunk period g, dw rows
            # 2g,2g+1 two periods later; final dw rows singly so the
            # last arrival feeds only the last 4 matmuls).
            dwq = {1: (0, 2), 2: (2, 4), 3: (4, 6), 4: (6, 8), 5: (8, 10),
                   6: (10, 12), 7: (12, 14)}
            for g in range(1, NCH):
                dma_gu(g, nc.sync)
                k0, k1 = dwq[g]
                nc.sync.dma_start(
                    dw_s[:, k0:k1, :], dw_v[:, k0 * DIM:k1 * DIM]
                )
            for k0, k1 in [(14, 15), (15, 16)]:
                nc.sync.dma_start(
                    dw_s[:, k0:k1, :], dw_v[:, k0 * DIM:k1 * DIM]
                )

            # --- chunk pipeline
            po = [
                psO.tile([128, 512], f32, tag="po", name=f"po{tok}_{dc}")
                for tok in range(NTOK) for dc in range(NDC)
            ]
            ht = [None] * NCH

            def gate_up(g):
                pg = psA.tile([128, PAIR, CPAD], f32, tag="pg")
                pu = psB.tile([128, PAIR, CPAD], f32, tag="pu")
                ht[g] = ht_pool.tile([128, PAIR, CPAD], f16, tag="ht", name=f"ht{g}")
                for j in range(PAIR):
                    cj = j * 128
                    for k in range(KC):
                        nc.tensor.matmul(
                            pg[:, j, :],
                            gu_s[:, g, 0, k * CH + cj:k * CH + cj + 128],
                            xt_s[:, k, :],
                            start=(k == 0), stop=(k == KC - 1),
                            skip_group_check=True,
                        )
                    for k in range(KC):
                        nc.tensor.matmul(
                            pu[:, j, :],
                            gu_s[:, g, 1, k * CH + cj:k * CH + cj + 128],
                            xt_s[:, k, :],
                            start=(k == 0), stop=(k == KC - 1),
                            skip_group_check=True,
                        )
                    stmp = stmp_pool.tile([128, CPAD], f32, tag="stmp",
                                          name=f"stmp{g}_{j}")
                    nc.scalar.activation(
                        stmp[:], pg[:, j, :], mybir.ActivationFunctionType.Silu
                    )
                    nc.vector.tensor_mul(ht[g][:, j, :], stmp[:], pu[:, j, :])

            def down(g):
                for k in (2 * g, 2 * g + 1):
                    for tok in range(NTOK):
                        t0, t1 = tok * 128, (tok + 1) * 128
                        for dc in range(NDC):
                            nc.tensor.matmul(
                                po[tok * NDC + dc][:],
                                ht[g][:, k % PAIR, t0:t1],
                                dw_s[:, k, dc * 512:(dc + 1) * 512],
                                start=(k == 0), stop=(k == KH - 1),
                                skip_group_check=True,
                            )

            for g in range(NCH):
                gate_up(g)
                if g >= 1:
                    down(g - 1)

            # Last down chunk, regrouped per chain: each chain's final
            # matmuls are followed immediately by its PSUM->fp16 cast and
            # output-quarter DMA, so they overlap the other chains' tail
            # matmuls and only ~1us of work remains after the last MM.
            g = NCH - 1
            out_tiles = [
                out_pool.tile([128, DIM], f16, tag="out", name=f"out{t}")
                for t in range(NTOK)
            ]
            for tok in range(NTOK):
                t0, t1 = tok * 128, (tok + 1) * 128
                for dc in range(NDC):
                    for k in (2 * g, 2 * g + 1):
                        nc.tensor.matmul(
                            po[tok * NDC + dc][:],
                            ht[g][:, k % PAIR, t0:t1],
                            dw_s[:, k, dc * 512:(dc + 1) * 512],
                            start=False, stop=(k == KH - 1),
                            skip_group_check=True,
                        )
                    nc.vector.tensor_copy(
                        out_tiles[tok][:, dc * 512:(dc + 1) * 512],
                        po[tok * NDC + dc][:],
                    )
                    nc.sync.dma_start(
                        o_d[t0:t1, dc * 512:(dc + 1) * 512],
                        out_tiles[tok][:, dc * 512:(dc + 1) * 512],
                    )

    nc.compile()
    return nc


def _get_nc():
    if "nc" not in _cache:
        _cache["nc"] = _build()
    return _cache["nc"]


def _pack_x(xe):
    # xe [CPAD, DIM] fp16 -> [128, KC*CPAD]: [p][k][c], dim = k*128+p
    return np.ascontiguousarray(
        xe.T.reshape(KC, 128, CPAD).transpose(1, 0, 2).reshape(128, KC * CPAD)
    )


def _pack_gu(gw, uw):
    # [DIM, HID] x2 fp16 -> [128, NCH*2*KC*CH]: [p][g][m][k][c], m=gate/up
    a = gw.reshape(KC, 128, NCH, CH).transpose(1, 2, 0, 3)  # [p][g][k][c]
    b = uw.reshape(KC, 128, NCH, CH).transpose(1, 2, 0, 3)
    return np.ascontiguousarray(
        np.stack([a, b], axis=2).reshape(128, -1)
    )


def _pack_dw(w):
    # w [HID, DIM] fp16 -> [128, KH*DIM]: [p][k][d]
    return np.ascontiguousarray(
        w.reshape(KH, 128, DIM).transpose(1, 0, 2).reshape(128, -1)
    )


def _run_block(nc, in_maps, collect):
    from concourse.bass_utils import run_bass_kernel_spmd

    kwargs = {} if collect is None else dict(collect.get("run_kwargs") or {})
    res = run_bass_kernel_spmd(nc, in_maps, core_ids=list(range(N_CORES)), **kwargs)
    if collect is not None:
        collect.setdefault("results", []).append(res)
    return [res.results[e]["o"] for e in range(E)]


def kernel(x, counts, gate_proj, up_proj, down_proj, _collect=None):
    x = np.asarray(x, dtype=np.float32).astype(F16)
    counts = np.asarray(counts, dtype=np.int32)
    gate_proj = np.asarray(gate_proj, dtype=np.float32).astype(F16)
    up_proj = np.asarray(up_proj, dtype=np.float32).astype(F16)
    down_proj = np.asarray(down_proj, dtype=np.float32).astype(F16)

    T = x.shape[0]
    offs = np.concatenate([[0], np.cumsum(counts)]).astype(np.int64)
    cmax = int(counts.max()) if counts.size else CPAD
    n_blocks = max(1, -(-cmax // CPAD))

    nc = _get_nc()
    wpacks = [
        {
            "gu": _pack_gu(gate_proj[e], up_proj[e]),
            "dw": _pack_dw(down_proj[e]),
        }
        for e in range(E)
    ]

    out = np.empty((T, DIM), dtype=np.float32)
    for b in range(n_blocks):
        in_maps = []
        spans = []
        for e in range(E):
            c = int(counts[e])
            s0 = min(b * CPAD, c)
            s1 = min((b + 1) * CPAD, c)
            xe = x[offs[e] + s0:offs[e] + s1]
            if xe.shape[0] < CPAD:
                xe = np.concatenate(
                    [xe, np.zeros((CPAD - xe.shape[0], DIM), F16)], axis=0
                )
            in_maps.append({"xt": _pack_x(xe), **wpacks[e]})
            spans.append((s0, s1))
        outs = _run_block(nc, in_maps, _collect)
        for e in range(E):
            s0, s1 = spans[e]
            if s1 > s0:
                out[offs[e] + s0:offs[e] + s1] = outs[e][: s1 - s0]
    return out
